# revision 1
# baseline (speedup 1.0000x reference)
"""BasicTransformerBlock on 8 TRN2 NeuronCores.

Sharding: sequence-parallel, zero collectives. The [B=2, N=2048, D=512]
residual stream is split into 8 row-blocks of 512 (4 cores per batch
element). Every core recomputes the cheap batch-wide work it needs
(adaln1 + K/V projections over its batch's 2048 rows, cond K/V), and does
attention / FFN only for its own 512 query rows.

Per-core inputs are pre-rotated with np.roll so that "own" rows are always
rows 0:512 -- the SPMD program is identical on all cores, only data differs.
Attention is permutation-invariant over keys, so rolled K/V is fine.

Layouts (SBUF tiles are [128 partitions, ...]):
  hT  = normed activations, transposed: [128 p=d%128, 4 dtile, rows] bf16
  kT  = [128 p=dout%128, 4 dtile, rows] bf16   (per head: 64 partitions)
  v   = [128 p=row%128, jt, 8 heads, 65] bf16  (col 64 of each head = 1.0,
        which makes the attention-weight row-sum (softmax denominator)
        fall out of the same matmul that computes attn@v)
Scores are computed transposed, sT[j, i], so exp() runs on ScalarE straight
out of PSUM and softmax normalization is applied per head on the tiny
attn@v result. Matmul operands are bf16 (weights are cast during the DMA
load by SWDGE); all accumulation/psum/residual math stays fp32.
"""

import contextlib

import numpy as np

import concourse.bass as bass
import concourse.mybir as mybir
import concourse.tile as tile
from concourse import bacc
from concourse.bass_utils import run_bass_kernel_spmd
from concourse.masks import make_identity

dt = mybir.dt
AF = mybir.ActivationFunctionType
OP = mybir.AluOpType

B, N, D = 2, 2048, 512
NCTX = 1024          # cond length
H = 8                # heads
HD = D // H          # 64
EPS = 1e-5
P = 128              # partitions
NCORES = 8
ROWS = 512           # own rows per core
NB = N               # batch rows per core (2048)
SCALE = HD ** -0.5   # 0.125

f32 = dt.float32
bf16 = dt.bfloat16

_CACHED = {}


def _adaln_stats(nc, stat_pool, src_tiles, n_tiles, eps_sb, chunk=4):
    """bn_stats/aggr + rstd/nmr for n_tiles row-tiles. Returns (rstd_all, nmr_all)."""
    mv_all = stat_pool.tile([P, n_tiles, 2], f32)
    rstd_all = stat_pool.tile([P, n_tiles], f32)
    nmr_all = stat_pool.tile([P, n_tiles], f32)
    for c0 in range(0, n_tiles, chunk):
        for it in range(c0, c0 + chunk):
            stats = stat_pool.tile([P, 6], f32, tag="stats")
            nc.vector.bn_stats(stats, src_tiles(it))
            nc.vector.bn_aggr(mv_all[:, it, :], stats)
        cs = slice(c0, c0 + chunk)
        nc.scalar.activation(rstd_all[:, cs], mv_all[:, cs, 1], AF.Sqrt,
                             bias=eps_sb, scale=1.0)
        nc.vector.reciprocal(rstd_all[:, cs], rstd_all[:, cs])
        nc.vector.scalar_tensor_tensor(
            nmr_all[:, cs], mv_all[:, cs, 0], -1.0, rstd_all[:, cs],
            op0=OP.mult, op1=OP.mult,
        )
    return rstd_all, nmr_all


def _adaln_apply(nc, tc, src_tiles, n_tiles, ab, rstd_all, nmr_all, hT,
                 ident_bf16, name):
    """xn = (x-mean)*rstd -> PE transpose -> fused (1+scale)/shift copy."""
    with contextlib.ExitStack() as actx:
        xn_pool = actx.enter_context(tc.tile_pool(name=f"{name}_xn", bufs=3))
        pst_pool = actx.enter_context(
            tc.tile_pool(name=f"{name}_pst", bufs=2, space="PSUM")
        )
        for it in range(n_tiles):
            xn = xn_pool.tile([P, 512], bf16, tag="xn")
            nc.scalar.activation(xn, src_tiles(it), AF.Identity,
                                 bias=nmr_all[:, it:it + 1],
                                 scale=rstd_all[:, it:it + 1])
            xnt = pst_pool.tile([P, 4, P], bf16, tag="xnt")
            for b in range(4):
                nc.tensor.transpose(
                    xnt[:, b, :], xn[:, b * P:(b + 1) * P], ident_bf16
                )
            for b in range(4):
                nc.vector.tensor_scalar(
                    hT[:, b, it * P:(it + 1) * P], xnt[:, b, :],
                    ab[:, b:b + 1], ab[:, 4 + b:5 + b],
                    op0=OP.mult, op1=OP.add,
                )


def _adaln_to_hT(nc, tc, src_tiles, n_tiles, ab, hT, ident_bf16, eps_sb, name):
    with contextlib.ExitStack() as actx:
        stat_pool = actx.enter_context(tc.tile_pool(name=f"{name}_stat", bufs=4))
        rstd_all, nmr_all = _adaln_stats(nc, stat_pool, src_tiles, n_tiles, eps_sb)
        _adaln_apply(nc, tc, src_tiles, n_tiles, ab, rstd_all, nmr_all, hT,
                     ident_bf16, name)


def _flush_av(nc, v, njt, pend, dn_pool, dnms):
    avp, h, jp, et = pend
    for sub in range(2):
        jt = jp * 2 + sub
        nc.tensor.matmul(
            avp, v[:, jt, h, :], et[:, sub, :],
            start=(jt == 0), stop=(jt == njt - 1),
        )
    if jp == njt // 2 - 1:
        # head finished: pull the denominator row out so the psum bank frees
        dnm_h = dn_pool.tile([1, 512], bf16, tag="dnm")
        nc.vector.tensor_copy(dnm_h, avp[HD:HD + 1, :])
        dnms[h] = dnm_h


def _attention(nc, tc, act, qT, kT, v, njt, wo, ob_row, ones_row,
               ident_bf16, x_res, x_out, name):
    """Transposed-score attention for 8 heads over own 512 rows.

    qT/kT: [128, 4, rows] bf16; v: [128, njt, 8, 65] bf16 (65th col ones).
    Writes x_out = attn_out @ wo + ob + x_res  (all [128, 4, 512] f32).
    """
    av_all = act.tile([P, 4, ROWS], bf16, tag="tH")
    with (
        tc.tile_pool(name=f"{name}_ps_s", bufs=2, space="PSUM") as ps_s,
        tc.tile_pool(name=f"{name}_ps_av", bufs=4, space="PSUM") as ps_av,
        tc.tile_pool(name=f"{name}_et", bufs=3) as et_pool,
        tc.tile_pool(name=f"{name}_dn", bufs=4) as dn_pool,
    ):
        # K=64/M=65 attention matmuls don't register as "busy" in the PE
        # activity monitor, so the clock gate holds the array at 1.2 GHz.
        # A full-array LDWEIGHTS every other step keeps it at 2.4 GHz
        # (the next real matmul reloads its own weights anyway).
        for grp in range(2):
            heads = range(grp * 4, grp * 4 + 4)
            avps = {}
            dnms = {}
            # software-pipelined: scores/exp for step n+1 issue before the
            # av matmuls of step n, so the in-order PE stream never waits on
            # ScalarE's exp latency. The skew also crosses head boundaries.
            pend = None   # (avp, h, jp, et)
            for h in heads:
                po = 64 * (h % 2)
                ht = h // 2
                avp = ps_av.tile([HD + 1, ROWS], f32, tag="av")
                avps[h] = avp
                for jp in range(njt // 2):
                    sps = ps_s.tile([P, 2, ROWS], f32, tag="s")
                    for sub in range(2):
                        jt = jp * 2 + sub
                        nc.tensor.matmul(
                            sps[:, sub, :],
                            kT[po:po + HD, ht, jt * P:(jt + 1) * P],
                            qT[po:po + HD, ht, :],
                            start=True, stop=True,
                        )
                    et = et_pool.tile([P, 2, ROWS], bf16, tag="et")
                    nc.scalar.activation(et, sps, AF.Exp, scale=SCALE)
                    if jp % 2 == 0:
                        nc.tensor.ldweights(ident_bf16)
                    if pend is not None:
                        _flush_av(nc, v, njt, pend, dn_pool, dnms)
                    pend = (avp, h, jp, et)
                # flush within-head at head end is deferred; pend carries over
            if pend is not None:
                _flush_av(nc, v, njt, pend, dn_pool, dnms)
            # broadcast denom rows across 64 partitions via K=1 matmuls,
            # then one full-width reciprocal per head-pair (psum -> sbuf)
            rbs = {}
            for pair in range(2):
                rb = ps_s.tile([P, 2, ROWS], f32, tag="s")
                for sub in range(2):
                    hh = grp * 4 + pair * 2 + sub
                    nc.tensor.matmul(
                        rb[sub * HD:(sub + 1) * HD, 0, :],
                        ones_row[0:1, 0:HD],
                        dnms[hh][0:1, :],
                        start=True, stop=True,
                    )
                rb_sb = et_pool.tile([P, ROWS], bf16, tag="rb")
                with nc.allow_low_precision(reason="bf16 softmax recip"):
                    nc.vector.reciprocal(rb_sb, rb[:, 0, :])
                rbs[pair] = rb_sb
            for h in heads:
                po = 64 * (h % 2)
                nc.vector.scalar_tensor_tensor(
                    av_all[po:po + HD, h // 2, :],
                    avps[h][0:HD, :], 1.0, rbs[(h % 4) // 2][po:po + HD, :],
                    op0=OP.mult, op1=OP.mult,
                )
    # out-projection + bias + residual
    with tc.tile_pool(name=f"{name}_ps_o", bufs=2, space="PSUM") as ps_o:
        for it in range(4):
            ps = ps_o.tile([P, D], f32, tag="o")
            for dt_ in range(4):
                nc.tensor.matmul(
                    ps, av_all[:, dt_, it * P:(it + 1) * P], wo[:, dt_, :],
                    start=(dt_ == 0), stop=False,
                )
            nc.tensor.matmul(
                ps, ones_row[0:1, 0:P], ob_row, start=False, stop=True,
            )
            nc.vector.tensor_tensor(x_out[:, it, :], ps, x_res[:, it, :], op=OP.add)


def build(max_phase=5):
    nc = bacc.Bacc(None, target_bir_lowering=False)

    # ---------------- I/O ----------------
    xb = nc.dram_tensor("xb", [NB, D], f32, kind="ExternalInput")
    condb = nc.dram_tensor("condb", [NCTX, D], f32, kind="ExternalInput")
    t_in = nc.dram_tensor("t", [D], f32, kind="ExternalInput")
    nw = {}
    nb_ = {}
    for l in (1, 2, 4):
        nw[l] = nc.dram_tensor(f"n{l}_w", [D, 2 * D], f32, kind="ExternalInput")
        nb_[l] = nc.dram_tensor(f"n{l}_b", [2 * D], f32, kind="ExternalInput")
    aw = {}
    for a in (1, 2):
        for w in "qkvo":
            aw[a, w] = nc.dram_tensor(f"a{a}_{w}", [D, D], f32, kind="ExternalInput")
        aw[a, "ob"] = nc.dram_tensor(f"a{a}_ob", [D], f32, kind="ExternalInput")
    ff_w1 = nc.dram_tensor("ff_w1", [D, 8 * D], f32, kind="ExternalInput")
    ff_b1 = nc.dram_tensor("ff_b1", [8 * D], f32, kind="ExternalInput")
    ff_w2 = nc.dram_tensor("ff_w2", [4 * D, D], f32, kind="ExternalInput")
    ff_b2 = nc.dram_tensor("ff_b2", [D], f32, kind="ExternalInput")
    out = nc.dram_tensor("out", [ROWS, D], f32, kind="ExternalOutput")

    with tile.TileContext(nc) as tc, contextlib.ExitStack() as ctx:
        const = ctx.enter_context(tc.tile_pool(name="const", bufs=1))
        wpool = ctx.enter_context(tc.tile_pool(name="wpool", bufs=1))
        act = ctx.enter_context(tc.tile_pool(name="act", bufs=1))

        ident_bf16 = const.tile([P, P], bf16)
        make_identity(nc, ident_bf16)
        ident_f32 = const.tile([P, P], f32)
        make_identity(nc, ident_f32)
        ones_row = const.tile([1, P], bf16)
        nc.vector.memset(ones_row, 1.0)
        eps_sb = const.tile([P, 1], f32)
        nc.vector.memset(eps_sb, EPS)

        # PE warmup: ~50 dependency-free matmuls fill the otherwise idle
        # startup window and lift the HAM clock gate to 2.4 GHz early
        with tc.tile_pool(name="warm", bufs=1, space="PSUM") as warm_pool:
            wps = warm_pool.tile([P, P], f32)
            for _ in range(50):
                nc.tensor.matmul(wps, ident_bf16, ident_bf16,
                                 start=True, stop=True)

        # t as column tiles [128, 4] bf16 for emb matmul lhsT
        tT = const.tile([P, 4], bf16)
        nc.gpsimd.dma_start(tT, t_in[:].rearrange("(k p) -> p k", p=P))

        h1T = act.tile([P, 4, NB], bf16, tag="tA")
        own_x = act.tile([P, 4, D], f32, tag="tE")
        xrest = act.tile([P, 12, D], f32, tag="tX")
        x_tiles = {}
        for it in range(16):
            dst = own_x[:, it, :] if it < 4 else xrest[:, it - 4, :]
            nc.sync.dma_start(dst, xb[:][it * P:(it + 1) * P, :])
            x_tiles[it] = dst

        # adaln1 stats issue first: independent of norm weights, keeps DVE
        # busy while the emb chain waits on its weight DMAs
        n1_stat = ctx.enter_context(tc.tile_pool(name="n1_stat", bufs=4))
        if max_phase >= 1:
            rstd1, nmr1 = _adaln_stats(nc, n1_stat, lambda it: x_tiles[it],
                                       16, eps_sb)

        # ---------------- norm scale/shift params ----------------
        # emb = t @ nw + nb  -> [1, 1024] -> dram bounce -> [128, 8] columns
        with (
            tc.tile_pool(name="nwp", bufs=1) as nwp,
            tc.tile_pool(name="embp", bufs=2) as embp,
            tc.tile_pool(name="ps_emb", bufs=2, space="PSUM") as ps_emb,
        ):
            ab = {}
            for l in (1, 2, 4):
                nw_sb = nwp.tile([P, 4, 2 * D], bf16, tag="nw")
                nc.gpsimd.dma_start(
                    nw_sb, nw[l][:].rearrange("(k p) n -> p k n", p=P)
                )
                nb_row = embp.tile([1, 2 * D], f32, tag="nbrow")
                nc.sync.dma_start(nb_row, nb_[l][:].rearrange("(a n) -> a n", a=1))
                emb_ps = ps_emb.tile([1, 2 * D], f32, tag="embps")
                for half in range(2):
                    for kt in range(4):
                        nc.tensor.matmul(
                            emb_ps[:, half * D:(half + 1) * D],
                            tT[:, kt:kt + 1],
                            nw_sb[:, kt, half * D:(half + 1) * D],
                            start=(kt == 0), stop=(kt == 3),
                        )
                emb_row = embp.tile([1, 2 * D], f32, tag="embrow")
                nc.vector.tensor_tensor(emb_row, emb_ps, nb_row, op=OP.add)
                # row -> per-partition columns via tiny PE transposes;
                # scale columns (0:4) get the +1 fused into the psum copy
                ab_l = const.tile([P, 8], f32, tag=f"ab{l}")
                for col in range(8):
                    tp = ps_emb.tile([P, 1], f32, tag="embT")
                    nc.tensor.transpose(
                        tp, emb_row[0:1, col * P:(col + 1) * P],
                        ident_f32[0:1, 0:1]
                    )
                    nc.vector.tensor_scalar(
                        ab_l[:, col:col + 1], tp,
                        1.0 if col < 4 else 0.0, None, op0=OP.add,
                    )
                ab[l] = ab_l



        # ---------------- attention weights (bf16 via DMA cast) ----------
        # a1 stack shares addresses with ff_w1, a2 stack with ff_w2
        # (sequential lifetimes; Tile inserts the WAR deps).
        a_sb = {}
        for a, wtag in ((1, "wbig1"), (2, "wbig2")):
            stack = wpool.tile([P, 4, 4, D], bf16, tag=wtag)
            for wi, w in enumerate("qkvo"):
                nc.gpsimd.dma_start(
                    stack[:, :, wi, :],
                    aw[a, w][:].rearrange("(k p) n -> p k n", p=P),
                )
                a_sb[a, w] = stack[:, :, wi, :]
            ob = wpool.tile([1, D], bf16, tag=f"a{a}ob")
            nc.gpsimd.dma_start(ob, aw[a, "ob"][:].rearrange("(a n) -> a n", a=1))
            a_sb[a, "ob"] = ob


        # ---------------- phase 1: adaln1 apply -> h1T -------------------
        if max_phase < 1:
            final = own_x
        else:
            _adaln_apply(nc, tc, lambda it: x_tiles[it], 16, ab[1], rstd1,
                         nmr1, h1T, ident_bf16, "n1")
            final = own_x

        # ---------------- phase 2: projections k1T, v1, q1T --------------
        if max_phase >= 2:
            k1T = act.tile([P, 4, NB], bf16, tag="tB")
            v1 = act.tile([P, 16, H, HD + 1], bf16, tag="tC")
            q1T = act.tile([P, 4, ROWS], bf16, tag="tD")
            nc.vector.memset(v1[:, :, :, HD:HD + 1], 1.0)
            with tc.tile_pool(name="ps_proj1", bufs=4, space="PSUM") as ps_proj:
                for dt_ in range(4):
                    for jc in range(4):
                        ps = ps_proj.tile([P, 512], f32, tag="proj")
                        for kt in range(4):
                            nc.tensor.matmul(
                                ps,
                                a_sb[1, "k"][:, kt, dt_ * P:(dt_ + 1) * P],
                                h1T[:, kt, jc * 512:(jc + 1) * 512],
                                start=(kt == 0), stop=(kt == 3),
                            )
                        nc.vector.tensor_copy(
                            k1T[:, dt_, jc * 512:(jc + 1) * 512], ps
                        )
                for jt in range(16):
                    ps = ps_proj.tile([P, 512], f32, tag="proj")
                    for kt in range(4):
                        nc.tensor.matmul(
                            ps,
                            h1T[:, kt, jt * P:(jt + 1) * P],
                            a_sb[1, "v"][:, kt, :],
                            start=(kt == 0), stop=(kt == 3),
                        )
                    nc.vector.tensor_copy(
                        v1[:, jt, :, 0:HD], ps.rearrange("p (h d) -> p h d", h=H)
                    )
                for dt_ in range(4):
                    ps = ps_proj.tile([P, 512], f32, tag="proj")
                    for kt in range(4):
                        nc.tensor.matmul(
                            ps,
                            a_sb[1, "q"][:, kt, dt_ * P:(dt_ + 1) * P],
                            h1T[:, kt, 0:ROWS],
                            start=(kt == 0), stop=(kt == 3),
                        )
                    nc.vector.tensor_copy(q1T[:, dt_, :], ps)

        # ------- early cross-attn prep: condT, k2T, v2 (independent of x) ----
        if max_phase >= 4:
            condT = act.tile([P, 4, NCTX], bf16, tag="tE2")
            with (
                tc.tile_pool(name="cin", bufs=3) as cin,
                tc.tile_pool(name="ps_ct", bufs=2, space="PSUM") as ps_ct,
            ):
                for it in range(8):
                    c_sb = cin.tile([P, D], f32, tag="ctile")
                    nc.sync.dma_start(c_sb, condb[:][it * P:(it + 1) * P, :])
                    ct = ps_ct.tile([P, 4, P], f32, tag="ct")
                    for b in range(4):
                        nc.tensor.transpose(
                            ct[:, b, :], c_sb[:, b * P:(b + 1) * P], ident_f32
                        )
                    for b in range(4):
                        nc.vector.tensor_copy(
                            condT[:, b, it * P:(it + 1) * P], ct[:, b, :]
                        )
            k2T = act.tile([P, 4, NCTX], bf16, tag="tX")
            v2 = act.tile([P, 8, H, HD + 1], bf16, tag="tI")
            nc.vector.memset(v2[:, :, :, HD:HD + 1], 1.0)
            with tc.tile_pool(name="ps_proj2a", bufs=4, space="PSUM") as ps_proj:
                for dt_ in range(4):
                    for jc in range(2):
                        ps = ps_proj.tile([P, 512], f32, tag="proj")
                        for kt in range(4):
                            nc.tensor.matmul(
                                ps,
                                a_sb[2, "k"][:, kt, dt_ * P:(dt_ + 1) * P],
                                condT[:, kt, jc * 512:(jc + 1) * 512],
                                start=(kt == 0), stop=(kt == 3),
                            )
                        nc.vector.tensor_copy(
                            k2T[:, dt_, jc * 512:(jc + 1) * 512], ps
                        )
                for jt in range(8):
                    ps = ps_proj.tile([P, 512], f32, tag="proj")
                    for kt in range(4):
                        nc.tensor.matmul(
                            ps,
                            condT[:, kt, jt * P:(jt + 1) * P],
                            a_sb[2, "v"][:, kt, :],
                            start=(kt == 0), stop=(kt == 3),
                        )
                    nc.vector.tensor_copy(
                        v2[:, jt, :, 0:HD], ps.rearrange("p (h d) -> p h d", h=H)
                    )

        # ---------------- phase 3: attention 1 ---------------------------
        if max_phase >= 3:
            x2 = act.tile([P, 4, D], f32, tag="tF")
            _attention(nc, tc, act, q1T, k1T, v1, 16, a_sb[1, "o"],
                       a_sb[1, "ob"], ones_row, ident_bf16, own_x, x2, "att1")
            final = x2

        # ---------------- phase 4: adaln2 + cross-attn -------------------
        if max_phase >= 4:
            h2T = act.tile([P, 4, ROWS], bf16, tag="tH")
            _adaln_to_hT(nc, tc, lambda it: x2[:, it, :], 4, ab[2], h2T,
                         ident_bf16, eps_sb, "n2")

            q2T = act.tile([P, 4, ROWS], bf16, tag="tJ")
            with tc.tile_pool(name="ps_proj2b", bufs=2, space="PSUM") as ps_proj:
                for dt_ in range(4):
                    ps = ps_proj.tile([P, 512], f32, tag="proj")
                    for kt in range(4):
                        nc.tensor.matmul(
                            ps,
                            a_sb[2, "q"][:, kt, dt_ * P:(dt_ + 1) * P],
                            h2T[:, kt, :],
                            start=(kt == 0), stop=(kt == 3),
                        )
                    nc.vector.tensor_copy(q2T[:, dt_, :], ps)

            x3 = act.tile([P, 4, D], f32, tag="tG")
            _attention(nc, tc, act, q2T, k2T, v2, 8, a_sb[2, "o"],
                       a_sb[2, "ob"], ones_row, ident_bf16, x2, x3, "att2")
            final = x3

        # ---------------- phase 5: adaln3 + GEGLU FFN --------------------
        if max_phase >= 5:
            h3T = act.tile([P, 4, ROWS], bf16, tag="tD")
            _adaln_to_hT(nc, tc, lambda it: x3[:, it, :], 4, ab[4], h3T,
                         ident_bf16, eps_sb, "n4")

            w1_sb = wpool.tile([P, 4, 8 * D], bf16, tag="wbig1")
            nc.gpsimd.dma_start(w1_sb, ff_w1[:].rearrange("(k p) n -> p k n", p=P))
            w2_sb = wpool.tile([P, 16, D], bf16, tag="wbig2")
            nc.gpsimd.dma_start(w2_sb, ff_w2[:].rearrange("(k p) n -> p k n", p=P))
            b1_sb = const.tile([P, 32], f32)
            nc.sync.dma_start(b1_sb, ff_b1[:].rearrange("(k p) -> p k", p=P))
            b2_row = const.tile([1, D], bf16)
            nc.gpsimd.dma_start(b2_row, ff_b2[:].rearrange("(a n) -> a n", a=1))

            ugT = act.tile([P, 16, ROWS], bf16, tag="tB")
            with (
                tc.tile_pool(name="ps_z", bufs=4, space="PSUM") as ps_z,
                tc.tile_pool(name="gact", bufs=3) as gact_pool,
            ):
                for ut in range(16):
                    zu = ps_z.tile([P, ROWS], f32, tag="z")
                    zg = ps_z.tile([P, ROWS], f32, tag="z")
                    for kt in range(4):
                        nc.tensor.matmul(
                            zu, w1_sb[:, kt, ut * P:(ut + 1) * P],
                            h3T[:, kt, :], start=(kt == 0), stop=(kt == 3),
                        )
                    for kt in range(4):
                        nc.tensor.matmul(
                            zg, w1_sb[:, kt, (16 + ut) * P:(17 + ut) * P],
                            h3T[:, kt, :], start=(kt == 0), stop=(kt == 3),
                        )
                    gact = gact_pool.tile([P, ROWS], bf16, tag="gact")
                    nc.scalar.activation(
                        gact, zg, AF.Gelu, bias=b1_sb[:, 16 + ut:17 + ut], scale=1.0
                    )
                    nc.vector.scalar_tensor_tensor(
                        ugT[:, ut, :], zu, b1_sb[:, ut:ut + 1], gact,
                        op0=OP.add, op1=OP.mult,
                    )

            out_sb = act.tile([P, 4, D], f32, tag="tC")
            with tc.tile_pool(name="ps_y", bufs=2, space="PSUM") as ps_y:
                for it in range(4):
                    ps = ps_y.tile([P, D], f32, tag="y")
                    for kt in range(16):
                        nc.tensor.matmul(
                            ps, ugT[:, kt, it * P:(it + 1) * P],
                            w2_sb[:, kt, :],
                            start=(kt == 0), stop=False,
                        )
                    nc.tensor.matmul(
                        ps, ones_row[0:1, 0:P], b2_row, start=False, stop=True,
                    )
                    nc.vector.tensor_tensor(
                        out_sb[:, it, :], ps, x3[:, it, :], op=OP.add
                    )
            final = out_sb

        for it_ in range(4):
            nc.sync.dma_start(out[:][it_ * P:(it_ + 1) * P, :], final[:, it_, :])

    nc.compile()
    return nc


def _shard_inputs(inputs):
    """Build the 8 per-core input maps."""
    x = np.ascontiguousarray(inputs["x"], dtype=np.float32)
    t = np.ascontiguousarray(inputs["t"], dtype=np.float32)
    cond = np.ascontiguousarray(inputs["cond"], dtype=np.float32)
    shared = {}
    for k in ("n1_w", "n1_b", "n2_w", "n2_b", "n4_w", "n4_b",
              "a1_q", "a1_k", "a1_v", "a1_o", "a1_ob",
              "a2_q", "a2_k", "a2_v", "a2_o", "a2_ob",
              "ff_w1", "ff_b1", "ff_w2", "ff_b2"):
        shared[k] = np.ascontiguousarray(inputs[k], dtype=np.float32)
    in_maps = []
    for c in range(NCORES):
        b = c // 4
        r0 = (c % 4) * ROWS
        m = dict(shared)
        m["xb"] = np.ascontiguousarray(np.roll(x[b], -r0, axis=0))
        m["condb"] = np.ascontiguousarray(cond[b])
        m["t"] = np.ascontiguousarray(t[b, 0])
        in_maps.append(m)
    return in_maps


def kernel(**inputs) -> np.ndarray:
    if "nc" not in _CACHED:
        _CACHED["nc"] = build()
    nc = _CACHED["nc"]
    in_maps = _shard_inputs(inputs)
    res = run_bass_kernel_spmd(nc, in_maps, core_ids=list(range(NCORES)))
    outs = [res.results[c]["out"] for c in range(NCORES)]
    full = np.concatenate(outs, axis=0).reshape(B, N, D)
    return full.astype(np.float32)

